# revision 2
# baseline (speedup 1.0000x reference)
"""Trainium2 Bass kernel for nn_BlurF: depthwise 4x4 blur (upfirdn2d pad=(2,1)).

Strategy: data-parallel over batch (8 cores x 1 image of [128,256,256]).
Per core, the separable conv is computed as two PE banded-matmul passes
using the data as the stationary operand, which transposes each pass:
  pass1: VT[x, y'] = sum_y X[y, x] * Bv[y, y']   (vertical conv, transposed)
  pass2: OUT[y', x'] = sum_x VT[x, y'] * Bh[x, x'] (horizontal conv, back)
Boundary zero-padding is folded into the band matrices. Matmuls run in
float32r (inputs rounded to 12-bit significand; accumulation is fp32 in
PSUM). General (non-separable) 4x4 kernels are handled via SVD as a sum
of up to 4 separable components.
"""

import numpy as np
import concourse.bacc as bacc
import concourse.mybir as mybir
from concourse.tile import TileContext
from concourse.bass_utils import run_bass_kernel_spmd

N_CORES = 8
C, H, W = 128, 256, 256
G = 8  # channels per DMA group
PRECISION = "fp32r"  # "fp32r" | "fp32"

_BUILD_CACHE = {}


def _round_f32r(a):
    """Round fp32 array to float32r (11 stored mantissa bits), round-half-up."""
    b = np.ascontiguousarray(a, dtype=np.float32).view(np.uint32)
    b = (b + np.uint32(0x800)) & np.uint32(0xFFFFF000)
    return b.view(np.float32)


def _factorize(kernel4x4):
    """kernel[a,b] = sum_r u_r[a] v_r[b]; returns list of (u, v) float64."""
    k = np.asarray(kernel4x4, dtype=np.float64)
    U, S, Vt = np.linalg.svd(k)
    comps = []
    for r in range(4):
        if S[r] > 1e-9 * max(S[0], 1e-30):
            comps.append((U[:, r] * np.sqrt(S[r]), Vt[r, :] * np.sqrt(S[r])))
    return comps


def _band(taps, n):
    """B[s, s'] = taps[a] where s = s' + 1 - a, for a in 0..3, clipped to [0,n)."""
    B = np.zeros((n, n), dtype=np.float64)
    for a in range(4):
        # s' = s + a - 1
        lo = max(0, 1 - a)
        hi = min(n, n + 1 - a)
        s = np.arange(lo, hi)
        B[s, s + a - 1] = taps[a]
    return B


def _emit(nc, tc, x, y, bvt, bht, rank, mmdt):
    f32 = mybir.dt.float32
    NG = C // G
    with (
        tc.tile_pool(name="xin", bufs=2) as xin_pool,
        tc.tile_pool(name="vt", bufs=3) as vt_pool,
        tc.tile_pool(name="yout", bufs=2) as yout_pool,
        tc.tile_pool(name="p1", bufs=3, space="PSUM") as p1_pool,
        tc.tile_pool(name="p2", bufs=3, space="PSUM") as p2_pool,
    ):
        pending = [None]

        def emit_pass2(p):
            vts, youts, j, g = p
            ops = [(r, m) for r in range(rank) for m in (0, 1)]
            for q in (0, 1):
                p2 = p2_pool.tile([128, 256], f32, tag="p2")
                for i, (r, m) in enumerate(ops):
                    nc.tensor.matmul(
                        p2[:],
                        vts[(r, m)][:, q * 128:(q + 1) * 128],
                        bht[r][m][:],
                        start=(i == 0),
                        stop=(i == len(ops) - 1),
                    )
                if q == 0:
                    nc.vector.tensor_copy(youts[q][:, j, :], p2[:])
                else:
                    nc.scalar.copy(youts[q][:, j, :], p2[:])
            if j == G - 1:
                for q in (0, 1):
                    nc.sync.dma_start(
                        out=y[g * G:(g + 1) * G, q * 128:(q + 1) * 128, :]
                        .rearrange("c y x -> y c x"),
                        in_=youts[q][:],
                    )

        for g in range(NG):
            xins = []
            for t in (0, 1):
                xt = xin_pool.tile([128, G, 256], mmdt, tag=f"xin{t}", name=f"xin{t}")
                nc.sync.dma_start(
                    out=xt[:],
                    in_=x[g * G:(g + 1) * G, t * 128:(t + 1) * 128, :]
                    .rearrange("c y x -> y c x"),
                )
                xins.append(xt)
            youts = {
                q: yout_pool.tile([128, G, 256], f32, tag=f"yout{q}", name=f"yout{q}")
                for q in (0, 1)
            }
            for j in range(G):
                vts = {}
                for m in (0, 1):
                    for r in range(rank):
                        p1 = p1_pool.tile([128, 256], f32, tag="p1")
                        for t in (0, 1):
                            nc.tensor.matmul(
                                p1[:],
                                xins[t][:, j, m * 128:(m + 1) * 128],
                                bvt[r][t][:],
                                start=(t == 0),
                                stop=(t == 1),
                            )
                        v = vt_pool.tile([128, 256], mmdt, tag=f"vt{m}_{r}", name=f"vt{m}_{r}")
                        if m == 0:
                            nc.vector.tensor_copy(v[:], p1[:])
                        else:
                            nc.scalar.copy(v[:], p1[:])
                        vts[(r, m)] = v
                if pending[0] is not None:
                    emit_pass2(pending[0])
                pending[0] = (vts, youts, j, g)
        emit_pass2(pending[0])


def _build(rank, precision, reps=1):
    key = (rank, precision, reps)
    if key in _BUILD_CACHE:
        return _BUILD_CACHE[key]
    f32 = mybir.dt.float32
    mmdt = mybir.dt.float32r if precision == "fp32r" else f32
    nc = bacc.Bacc("TRN2", target_bir_lowering=False, debug=False)
    x = nc.dram_tensor("x", [C, H, W], mmdt, kind="ExternalInput").ap()
    bv = nc.dram_tensor("bv", [rank, 2, 128, 256], mmdt, kind="ExternalInput").ap()
    bh = nc.dram_tensor("bh", [rank, 2, 128, 256], mmdt, kind="ExternalInput").ap()
    y = nc.dram_tensor("y", [C, H, W], f32, kind="ExternalOutput").ap()
    with TileContext(nc) as tc:
        with tc.tile_pool(name="bands", bufs=1) as band_pool:
            bvt = [[None, None] for _ in range(rank)]
            bht = [[None, None] for _ in range(rank)]
            for r in range(rank):
                for t in (0, 1):
                    bvt[r][t] = band_pool.tile([128, 256], mmdt, tag=f"bv{r}{t}", name=f"bv{r}{t}")
                    nc.sync.dma_start(out=bvt[r][t][:], in_=bv[r, t])
                    bht[r][t] = band_pool.tile([128, 256], mmdt, tag=f"bh{r}{t}", name=f"bh{r}{t}")
                    nc.sync.dma_start(out=bht[r][t][:], in_=bh[r, t])
            for _ in range(reps):
                _emit(nc, tc, x, y, bvt, bht, rank, mmdt)
    nc.compile()
    _BUILD_CACHE[key] = nc
    return nc


def _prep_inputs(fmap, kernel4x4, precision):
    comps = _factorize(kernel4x4)
    rank = max(1, len(comps))
    while len(comps) < rank:
        comps.append((np.zeros(4), np.zeros(4)))
    bv = np.zeros((rank, 2, 128, 256), dtype=np.float32)
    bh = np.zeros((rank, 2, 128, 256), dtype=np.float32)
    for r, (u, v) in enumerate(comps):
        Bv = _band(u, H).astype(np.float32)  # [y, y']
        Bh = _band(v, W).astype(np.float32)  # [x, x']
        bv[r] = Bv.reshape(2, 128, 256)
        bh[r] = Bh.reshape(2, 128, 256)
    if precision == "fp32r":
        bv, bh = _round_f32r(bv), _round_f32r(bh)
    in_maps = []
    for i in range(N_CORES):
        shard = np.ascontiguousarray(fmap[i], dtype=np.float32)
        if precision == "fp32r":
            shard = _round_f32r(shard)
        in_maps.append({"x": shard, "bv": bv, "bh": bh})
    return rank, in_maps


def kernel(fmap, kernel):
    fmap = np.asarray(fmap)
    kern = np.asarray(kernel)
    assert fmap.shape == (N_CORES, C, H, W), fmap.shape
    rank, in_maps = _prep_inputs(fmap, kern, PRECISION)
    nc = _build(rank, PRECISION)
    res = run_bass_kernel_spmd(nc, in_maps, list(range(N_CORES)), trace=False)
    out = np.stack([res.results[i]["y"] for i in range(N_CORES)], axis=0)
    return np.ascontiguousarray(out, dtype=np.float32)


# revision 8
# speedup vs baseline: 1610.6117x; 1610.6117x over previous
"""Trainium2 Bass kernel for nn_BlurF: depthwise 4x4 blur (upfirdn2d pad=(2,1)).

Strategy: data-parallel over batch (8 cores x 1 image of [128,256,256]).
Per core, the separable conv is computed as two PE banded-matmul passes
using the data as the stationary operand, which transposes each pass:
  pass1: VT[x, y'] = sum_y X[y, x] * Bv[y, y']   (vertical conv, transposed)
  pass2: OUT[y', x'] = sum_x VT[x, y'] * Bh[x, x'] (horizontal conv, back)
Boundary zero-padding is folded into the band matrices. Matmuls run in
float32r (inputs rounded to 12-bit significand; accumulation is fp32 in
PSUM). General (non-separable) 4x4 kernels are handled via SVD as a sum
of up to 4 separable components.
"""

import numpy as np
import concourse.bacc as bacc
import concourse.mybir as mybir
from concourse.tile import TileContext
from concourse.bass_utils import run_bass_kernel_spmd

N_CORES = 8
C, H, W = 128, 256, 256
G = 8  # channels per DMA group
PRECISION = "fp32r"  # "fp32r" | "fp32"

_BUILD_CACHE = {}


def _round_f32r(a):
    """Round fp32 array to float32r (11 stored mantissa bits), round-half-up."""
    b = np.ascontiguousarray(a, dtype=np.float32).view(np.uint32)
    b = (b + np.uint32(0x800)) & np.uint32(0xFFFFF000)
    return b.view(np.float32)


def _factorize(kernel4x4):
    """kernel[a,b] = sum_r u_r[a] v_r[b]; returns list of (u, v) float64."""
    k = np.asarray(kernel4x4, dtype=np.float64)
    U, S, Vt = np.linalg.svd(k)
    comps = []
    for r in range(4):
        if S[r] > 1e-9 * max(S[0], 1e-30):
            comps.append((U[:, r] * np.sqrt(S[r]), Vt[r, :] * np.sqrt(S[r])))
    return comps


def _band(taps, n):
    """B[s, s'] = taps[a] where s = s' + 1 - a, for a in 0..3, clipped to [0,n)."""
    B = np.zeros((n, n), dtype=np.float64)
    for a in range(4):
        # s' = s + a - 1
        lo = max(0, 1 - a)
        hi = min(n, n + 1 - a)
        s = np.arange(lo, hi)
        B[s, s + a - 1] = taps[a]
    return B


DEFAULT_CFG = dict(
    G=16, fused_in=False, out_engine="scalar",
    xin_bufs=2, vt_bufs=3, yout_bufs=2, p1_bufs=3, p2_bufs=3,
)


def _emit(nc, tc, x, y, bvt, bht, rank, mmdt, cfg=None):
    cfg = {**DEFAULT_CFG, **(cfg or {})}
    Gc = cfg["G"]
    f32 = mybir.dt.float32
    NG = C // Gc
    out_dma = nc.scalar if cfg["out_engine"] == "scalar" else nc.sync
    with (
        tc.tile_pool(name="xin", bufs=cfg["xin_bufs"]) as xin_pool,
        tc.tile_pool(name="vt", bufs=cfg["vt_bufs"]) as vt_pool,
        tc.tile_pool(name="yout", bufs=cfg["yout_bufs"]) as yout_pool,
        tc.tile_pool(name="p1", bufs=cfg["p1_bufs"], space="PSUM") as p1_pool,
        tc.tile_pool(name="p2", bufs=cfg["p2_bufs"], space="PSUM") as p2_pool,
    ):
        pending = [None]

        def emit_pass2(p):
            vts, youts, j, g = p
            ops = [(r, m) for r in range(rank) for m in (0, 1)]
            for q in (0, 1):
                p2 = p2_pool.tile([128, 256], f32, tag="p2")
                for i, (r, m) in enumerate(ops):
                    nc.tensor.matmul(
                        p2[:],
                        vts[(r, m)][:, q * 128:(q + 1) * 128],
                        bht[r][m][:],
                        start=(i == 0),
                        stop=(i == len(ops) - 1),
                    )
                if q == 0:
                    nc.vector.tensor_copy(youts[q][:, j, :], p2[:])
                else:
                    nc.scalar.copy(youts[q][:, j, :], p2[:])
            if j == Gc - 1:
                for q in (0, 1):
                    out_dma.dma_start(
                        out=y[g * Gc:(g + 1) * Gc, q * 128:(q + 1) * 128, :]
                        .rearrange("c y x -> y c x"),
                        in_=youts[q][:],
                    )

        for g in range(NG):
            if cfg["fused_in"]:
                xw = xin_pool.tile([128, 2, Gc, 256], mmdt, tag="xin", name="xin")
                nc.sync.dma_start(
                    out=xw[:],
                    in_=x[g * Gc:(g + 1) * Gc, :, :]
                    .rearrange("c (t y) x -> y t c x", t=2),
                )
                xins = [xw[:, 0], xw[:, 1]]
            else:
                xins = []
                for t in (0, 1):
                    xt = xin_pool.tile([128, Gc, 256], mmdt, tag=f"xin{t}", name=f"xin{t}")
                    nc.sync.dma_start(
                        out=xt[:],
                        in_=x[g * Gc:(g + 1) * Gc, t * 128:(t + 1) * 128, :]
                        .rearrange("c y x -> y c x"),
                    )
                    xins.append(xt)
            youts = {
                q: yout_pool.tile([128, Gc, 256], f32, tag=f"yout{q}", name=f"yout{q}")
                for q in (0, 1)
            }
            for j in range(Gc):
                vts = {}
                for m in (0, 1):
                    for r in range(rank):
                        p1 = p1_pool.tile([128, 256], f32, tag="p1")
                        for t in (0, 1):
                            nc.tensor.matmul(
                                p1[:],
                                xins[t][:, j, m * 128:(m + 1) * 128],
                                bvt[r][t][:],
                                start=(t == 0),
                                stop=(t == 1),
                            )
                        v = vt_pool.tile([128, 256], mmdt, tag=f"vt{m}_{r}", name=f"vt{m}_{r}")
                        if m == 0:
                            nc.vector.tensor_copy(v[:], p1[:])
                        else:
                            nc.scalar.copy(v[:], p1[:])
                        vts[(r, m)] = v
                if pending[0] is not None:
                    emit_pass2(pending[0])
                pending[0] = (vts, youts, j, g)
        emit_pass2(pending[0])


def _build(rank, precision, reps=1, loop_reps=None, cfg=None):
    key = (rank, precision, reps, loop_reps,
           tuple(sorted((cfg or {}).items())))
    if key in _BUILD_CACHE:
        return _BUILD_CACHE[key]
    f32 = mybir.dt.float32
    mmdt = mybir.dt.float32r if precision == "fp32r" else f32
    nc = bacc.Bacc("TRN2", target_bir_lowering=False, debug=False)
    x = nc.dram_tensor("x", [C, H, W], mmdt, kind="ExternalInput").ap()
    bv = nc.dram_tensor("bv", [rank, 2, 128, 256], mmdt, kind="ExternalInput").ap()
    bh = nc.dram_tensor("bh", [rank, 2, 128, 256], mmdt, kind="ExternalInput").ap()
    y = nc.dram_tensor("y", [C, H, W], f32, kind="ExternalOutput").ap()
    with TileContext(nc) as tc:
        with tc.tile_pool(name="bands", bufs=1) as band_pool:
            bvt = [[None, None] for _ in range(rank)]
            bht = [[None, None] for _ in range(rank)]
            for r in range(rank):
                for t in (0, 1):
                    bvt[r][t] = band_pool.tile([128, 256], mmdt, tag=f"bv{r}{t}", name=f"bv{r}{t}")
                    nc.sync.dma_start(out=bvt[r][t][:], in_=bv[r, t])
                    bht[r][t] = band_pool.tile([128, 256], mmdt, tag=f"bh{r}{t}", name=f"bh{r}{t}")
                    nc.sync.dma_start(out=bht[r][t][:], in_=bh[r, t])
            if loop_reps is not None:
                with tc.For_i(0, loop_reps, 1):
                    _emit(nc, tc, x, y, bvt, bht, rank, mmdt, cfg)
            else:
                for _ in range(reps):
                    _emit(nc, tc, x, y, bvt, bht, rank, mmdt, cfg)
    nc.compile()
    _BUILD_CACHE[key] = nc
    return nc


def _prep_inputs(fmap, kernel4x4, precision):
    comps = _factorize(kernel4x4)
    rank = max(1, len(comps))
    while len(comps) < rank:
        comps.append((np.zeros(4), np.zeros(4)))
    bv = np.zeros((rank, 2, 128, 256), dtype=np.float32)
    bh = np.zeros((rank, 2, 128, 256), dtype=np.float32)
    for r, (u, v) in enumerate(comps):
        Bv = _band(u, H).astype(np.float32)  # [y, y']
        Bh = _band(v, W).astype(np.float32)  # [x, x']
        bv[r] = Bv.reshape(2, 128, 256)
        bh[r] = Bh.reshape(2, 128, 256)
    if precision == "fp32r":
        bv, bh = _round_f32r(bv), _round_f32r(bh)
    in_maps = []
    for i in range(N_CORES):
        shard = np.ascontiguousarray(fmap[i], dtype=np.float32)
        if precision == "fp32r":
            shard = _round_f32r(shard)
        in_maps.append({"x": shard, "bv": bv, "bh": bh})
    return rank, in_maps


def kernel(fmap, kernel):
    fmap = np.asarray(fmap)
    kern = np.asarray(kernel)
    assert fmap.shape == (N_CORES, C, H, W), fmap.shape
    rank, in_maps = _prep_inputs(fmap, kern, PRECISION)
    nc = _build(rank, PRECISION)
    last_err = None
    for _attempt in range(3):
        try:
            res = run_bass_kernel_spmd(nc, in_maps, list(range(N_CORES)), trace=False)
            break
        except Exception as e:  # transient device wedge -> retry
            last_err = e
    else:
        raise last_err
    out = np.stack([res.results[i]["y"] for i in range(N_CORES)], axis=0)
    return np.ascontiguousarray(out, dtype=np.float32)


# revision 10
# speedup vs baseline: 1656.2053x; 1.0283x over previous
"""Trainium2 Bass kernel for nn_BlurF: depthwise 4x4 blur (upfirdn2d pad=(2,1)).

Strategy: data-parallel over batch (8 cores x 1 image of [128,256,256]).
Per core, the separable conv is computed as two PE banded-matmul passes
using the data as the stationary operand, which transposes each pass:
  pass1: VT[x, y'] = sum_y X[y, x] * Bv[y, y']   (vertical conv, transposed)
  pass2: OUT[y', x'] = sum_x VT[x, y'] * Bh[x, x'] (horizontal conv, back)
Boundary zero-padding is folded into the band matrices. Matmuls run in
float32r (inputs rounded to 12-bit significand; accumulation is fp32 in
PSUM). General (non-separable) 4x4 kernels are handled via SVD as a sum
of up to 4 separable components.
"""

import numpy as np
import concourse.bacc as bacc
import concourse.mybir as mybir
from concourse.tile import TileContext
from concourse.bass_utils import run_bass_kernel_spmd

N_CORES = 8
C, H, W = 128, 256, 256
G = 8  # channels per DMA group
PRECISION = "fp32r"  # "fp32r" | "fp32"

_BUILD_CACHE = {}


def _round_f32r(a):
    """Round fp32 array to float32r (11 stored mantissa bits), round-half-up."""
    b = np.ascontiguousarray(a, dtype=np.float32).view(np.uint32)
    b = (b + np.uint32(0x800)) & np.uint32(0xFFFFF000)
    return b.view(np.float32)


def _factorize(kernel4x4):
    """kernel[a,b] = sum_r u_r[a] v_r[b]; returns list of (u, v) float64."""
    k = np.asarray(kernel4x4, dtype=np.float64)
    U, S, Vt = np.linalg.svd(k)
    comps = []
    for r in range(4):
        if S[r] > 1e-9 * max(S[0], 1e-30):
            comps.append((U[:, r] * np.sqrt(S[r]), Vt[r, :] * np.sqrt(S[r])))
    return comps


def _band(taps, n):
    """B[s, s'] = taps[a] where s = s' + 1 - a, for a in 0..3, clipped to [0,n)."""
    B = np.zeros((n, n), dtype=np.float64)
    for a in range(4):
        # s' = s + a - 1
        lo = max(0, 1 - a)
        hi = min(n, n + 1 - a)
        s = np.arange(lo, hi)
        B[s, s + a - 1] = taps[a]
    return B


DEFAULT_CFG = dict(
    G=16, fused_in=False, out_engine="scalar",
    xin_bufs=2, vt_bufs=3, yout_bufs=2, p1_bufs=3, p2_bufs=3,
)


def _emit(nc, tc, x, y, bvt, bht, rank, precision, cfg=None):
    cfg = {**DEFAULT_CFG, **(cfg or {})}
    Gc = cfg["G"]
    f32 = mybir.dt.float32
    f32r = mybir.dt.float32r
    mmdt = f32 if precision == "fp32" else f32r
    split = precision == "fp32r_split"
    parts = (0, 1) if split else (0,)
    NG = C // Gc
    out_dma = nc.scalar if cfg["out_engine"] == "scalar" else nc.sync
    with (
        tc.tile_pool(name="xin", bufs=cfg["xin_bufs"]) as xin_pool,
        tc.tile_pool(name="vt", bufs=cfg["vt_bufs"]) as vt_pool,
        tc.tile_pool(name="yout", bufs=cfg["yout_bufs"]) as yout_pool,
        tc.tile_pool(name="p1", bufs=cfg["p1_bufs"], space="PSUM") as p1_pool,
        tc.tile_pool(name="p2", bufs=cfg["p2_bufs"], space="PSUM") as p2_pool,
    ):
        pending = [None]

        def emit_pass2(p):
            vts, youts, j, g = p
            ops = [(r, m, s) for r in range(rank) for m in (0, 1) for s in parts]
            for q in (0, 1):
                p2 = p2_pool.tile([128, 256], f32, tag="p2")
                for i, (r, m, s) in enumerate(ops):
                    nc.tensor.matmul(
                        p2[:],
                        vts[(r, m, s)][:, q * 128:(q + 1) * 128],
                        bht[r][m][:],
                        start=(i == 0),
                        stop=(i == len(ops) - 1),
                    )
                if q == 0:
                    nc.vector.tensor_copy(youts[q][:, j, :], p2[:])
                else:
                    nc.scalar.copy(youts[q][:, j, :], p2[:])
            if j == Gc - 1:
                for q in (0, 1):
                    out_dma.dma_start(
                        out=y[g * Gc:(g + 1) * Gc, q * 128:(q + 1) * 128, :]
                        .rearrange("c y x -> y c x"),
                        in_=youts[q][:],
                    )

        for g in range(NG):
            xraw = []
            for t in (0, 1):
                xt = xin_pool.tile([128, Gc, 256], f32 if split else mmdt,
                                   tag=f"xin{t}", name=f"xin{t}")
                nc.sync.dma_start(
                    out=xt[:],
                    in_=x[g * Gc:(g + 1) * Gc, t * 128:(t + 1) * 128, :]
                    .rearrange("c y x -> y c x"),
                )
                xraw.append(xt)
            if split:
                # device-side hi/lo decomposition: x = hi + lo, both f32r
                xins = {}
                for t in (0, 1):
                    hi = xin_pool.tile([128, Gc, 256], f32r, tag=f"xhi{t}", name=f"xhi{t}")
                    nc.scalar.copy(hi[:], xraw[t][:])
                    lo = xin_pool.tile([128, Gc, 256], f32r, tag=f"xlo{t}", name=f"xlo{t}")
                    nc.vector.tensor_sub(lo[:], xraw[t][:], hi[:])
                    xins[(t, 0)] = hi
                    xins[(t, 1)] = lo
            else:
                xins = {(t, 0): xraw[t] for t in (0, 1)}
            youts = {
                q: yout_pool.tile([128, Gc, 256], f32, tag=f"yout{q}", name=f"yout{q}")
                for q in (0, 1)
            }
            for j in range(Gc):
                vts = {}
                for m in (0, 1):
                    for r in range(rank):
                        p1 = p1_pool.tile([128, 256], f32, tag="p1")
                        mmops = [(t, s) for t in (0, 1) for s in parts]
                        for i, (t, s) in enumerate(mmops):
                            nc.tensor.matmul(
                                p1[:],
                                xins[(t, s)][:, j, m * 128:(m + 1) * 128],
                                bvt[r][t][:],
                                start=(i == 0),
                                stop=(i == len(mmops) - 1),
                            )
                        if split:
                            vhi = vt_pool.tile([128, 256], f32r,
                                               tag=f"vth{m}_{r}", name=f"vth{m}_{r}")
                            nc.scalar.copy(vhi[:], p1[:])
                            vlo = vt_pool.tile([128, 256], f32r,
                                               tag=f"vtl{m}_{r}", name=f"vtl{m}_{r}")
                            nc.vector.tensor_sub(vlo[:], p1[:], vhi[:])
                            vts[(r, m, 0)] = vhi
                            vts[(r, m, 1)] = vlo
                        else:
                            v = vt_pool.tile([128, 256], mmdt,
                                             tag=f"vt{m}_{r}", name=f"vt{m}_{r}")
                            if m == 0:
                                nc.vector.tensor_copy(v[:], p1[:])
                            else:
                                nc.scalar.copy(v[:], p1[:])
                            vts[(r, m, 0)] = v
                if pending[0] is not None:
                    emit_pass2(pending[0])
                pending[0] = (vts, youts, j, g)
        emit_pass2(pending[0])


def _build(rank, precision, reps=1, loop_reps=None, cfg=None):
    key = (rank, precision, reps, loop_reps,
           tuple(sorted((cfg or {}).items())))
    if key in _BUILD_CACHE:
        return _BUILD_CACHE[key]
    f32 = mybir.dt.float32
    mmdt = f32 if precision == "fp32" else mybir.dt.float32r
    xdt = f32 if precision in ("fp32", "fp32r_split") else mmdt
    nc = bacc.Bacc("TRN2", target_bir_lowering=False, debug=False)
    x = nc.dram_tensor("x", [C, H, W], xdt, kind="ExternalInput").ap()
    bv = nc.dram_tensor("bv", [rank, 2, 128, 256], mmdt, kind="ExternalInput").ap()
    bh = nc.dram_tensor("bh", [rank, 2, 128, 256], mmdt, kind="ExternalInput").ap()
    y = nc.dram_tensor("y", [C, H, W], f32, kind="ExternalOutput").ap()
    with TileContext(nc) as tc:
        with tc.tile_pool(name="bands", bufs=1) as band_pool:
            bvt = [[None, None] for _ in range(rank)]
            bht = [[None, None] for _ in range(rank)]
            for r in range(rank):
                for t in (0, 1):
                    bvt[r][t] = band_pool.tile([128, 256], mmdt, tag=f"bv{r}{t}", name=f"bv{r}{t}")
                    nc.sync.dma_start(out=bvt[r][t][:], in_=bv[r, t])
                    bht[r][t] = band_pool.tile([128, 256], mmdt, tag=f"bh{r}{t}", name=f"bh{r}{t}")
                    nc.sync.dma_start(out=bht[r][t][:], in_=bh[r, t])
            if loop_reps is not None:
                with tc.For_i(0, loop_reps, 1):
                    _emit(nc, tc, x, y, bvt, bht, rank, precision, cfg)
            else:
                for _ in range(reps):
                    _emit(nc, tc, x, y, bvt, bht, rank, precision, cfg)
    nc.compile()
    _BUILD_CACHE[key] = nc
    return nc


def _prep_inputs(fmap, kernel4x4, precision):
    comps = _factorize(kernel4x4)
    rank = max(1, len(comps))
    while len(comps) < rank:
        comps.append((np.zeros(4), np.zeros(4)))
    bv = np.zeros((rank, 2, 128, 256), dtype=np.float32)
    bh = np.zeros((rank, 2, 128, 256), dtype=np.float32)
    for r, (u, v) in enumerate(comps):
        Bv = _band(u, H).astype(np.float32)  # [y, y']
        Bh = _band(v, W).astype(np.float32)  # [x, x']
        bv[r] = Bv.reshape(2, 128, 256)
        bh[r] = Bh.reshape(2, 128, 256)
    if precision in ("fp32r", "fp32r_split"):
        bv, bh = _round_f32r(bv), _round_f32r(bh)
    in_maps = []
    for i in range(N_CORES):
        shard = np.ascontiguousarray(fmap[i], dtype=np.float32)
        if precision == "fp32r":
            shard = _round_f32r(shard)

        in_maps.append({"x": shard, "bv": bv, "bh": bh})
    return rank, in_maps


def kernel(fmap, kernel):
    fmap = np.asarray(fmap)
    kern = np.asarray(kernel)
    assert fmap.shape == (N_CORES, C, H, W), fmap.shape
    rank, in_maps = _prep_inputs(fmap, kern, PRECISION)
    nc = _build(rank, PRECISION)
    last_err = None
    for _attempt in range(3):
        try:
            res = run_bass_kernel_spmd(nc, in_maps, list(range(N_CORES)), trace=False)
            break
        except Exception as e:  # transient device wedge -> retry
            last_err = e
    else:
        raise last_err
    out = np.stack([res.results[i]["y"] for i in range(N_CORES)], axis=0)
    return np.ascontiguousarray(out, dtype=np.float32)


# revision 12
# speedup vs baseline: 1688.6633x; 1.0196x over previous
"""Trainium2 Bass kernel for nn_BlurF: depthwise 4x4 blur (upfirdn2d pad=(2,1)).

Strategy: data-parallel over batch (8 cores x 1 image of [128,256,256]).
Per core, the separable conv is computed as two PE banded-matmul passes
using the data as the stationary operand, which transposes each pass:
  pass1: VT[x, y'] = sum_y X[y, x] * Bv[y, y']   (vertical conv, transposed)
  pass2: OUT[y', x'] = sum_x VT[x, y'] * Bh[x, x'] (horizontal conv, back)
Boundary zero-padding is folded into the band matrices. Matmuls run in
float32r (inputs rounded to 12-bit significand; accumulation is fp32 in
PSUM). General (non-separable) 4x4 kernels are handled via SVD as a sum
of up to 4 separable components.
"""

import numpy as np
import concourse.bacc as bacc
import concourse.mybir as mybir
from concourse.tile import TileContext
from concourse.bass_utils import run_bass_kernel_spmd

N_CORES = 8
C, H, W = 128, 256, 256
PRECISION = "fp32r"  # "fp32r" (fast, ~1.3e-4 rel err) | "fp32r_split" (hi/lo, ~6e-8) | "fp32"

_BUILD_CACHE = {}


def _round_f32r(a):
    """Round fp32 array to float32r (11 stored mantissa bits), round-half-up."""
    b = np.ascontiguousarray(a, dtype=np.float32).view(np.uint32)
    b = (b + np.uint32(0x800)) & np.uint32(0xFFFFF000)
    return b.view(np.float32)


def _factorize(kernel4x4):
    """kernel[a,b] = sum_r u_r[a] v_r[b]; returns list of (u, v) float64."""
    k = np.asarray(kernel4x4, dtype=np.float64)
    U, S, Vt = np.linalg.svd(k)
    comps = []
    for r in range(4):
        if S[r] > 1e-9 * max(S[0], 1e-30):
            comps.append((U[:, r] * np.sqrt(S[r]), Vt[r, :] * np.sqrt(S[r])))
    return comps


def _band(taps, n):
    """B[s, s'] = taps[a] where s = s' + 1 - a, for a in 0..3, clipped to [0,n)."""
    B = np.zeros((n, n), dtype=np.float64)
    for a in range(4):
        # s' = s + a - 1
        lo = max(0, 1 - a)
        hi = min(n, n + 1 - a)
        s = np.arange(lo, hi)
        B[s, s + a - 1] = taps[a]
    return B


DEFAULT_CFG = dict(
    G=16, out_engine="scalar",
    xin_bufs=2, vt_bufs=3, yout_bufs=2, p1_bufs=3, p2_bufs=3,
)


def _emit(nc, tc, x, y, bvt, bht, rank, precision, cfg=None):
    cfg = {**DEFAULT_CFG, **(cfg or {})}
    Gc = cfg["G"]
    f32 = mybir.dt.float32
    f32r = mybir.dt.float32r
    mmdt = f32 if precision == "fp32" else f32r
    split = precision == "fp32r_split"
    parts = (0, 1) if split else (0,)
    NG = C // Gc
    out_dma = nc.scalar if cfg["out_engine"] == "scalar" else nc.sync
    with (
        tc.tile_pool(name="xin", bufs=cfg["xin_bufs"]) as xin_pool,
        tc.tile_pool(name="vt", bufs=cfg["vt_bufs"]) as vt_pool,
        tc.tile_pool(name="yout", bufs=cfg["yout_bufs"]) as yout_pool,
        tc.tile_pool(name="p1", bufs=cfg["p1_bufs"], space="PSUM") as p1_pool,
        tc.tile_pool(name="p2", bufs=cfg["p2_bufs"], space="PSUM") as p2_pool,
    ):
        pending = [None]

        def emit_pass2(p):
            vts, youts, j, g = p
            ops = [(r, m, s) for r in range(rank) for m in (0, 1) for s in parts]
            for q in (0, 1):
                p2 = p2_pool.tile([128, 256], f32, tag="p2")
                for i, (r, m, s) in enumerate(ops):
                    nc.tensor.matmul(
                        p2[:],
                        vts[(r, m, s)][:, q * 128:(q + 1) * 128],
                        bht[r][m][:],
                        start=(i == 0),
                        stop=(i == len(ops) - 1),
                    )
                if q == 0:
                    nc.vector.tensor_copy(youts[q][:, j, :], p2[:])
                else:
                    nc.scalar.copy(youts[q][:, j, :], p2[:])
            if j == Gc - 1:
                for q in (0, 1):
                    out_dma.dma_start(
                        out=y[g * Gc:(g + 1) * Gc, q * 128:(q + 1) * 128, :]
                        .rearrange("c y x -> y c x"),
                        in_=youts[q][:],
                    )

        for g in range(NG):
            xraw = []
            for t in (0, 1):
                xt = xin_pool.tile([128, Gc, 256], f32 if split else mmdt,
                                   tag=f"xin{t}", name=f"xin{t}")
                nc.sync.dma_start(
                    out=xt[:],
                    in_=x[g * Gc:(g + 1) * Gc, t * 128:(t + 1) * 128, :]
                    .rearrange("c y x -> y c x"),
                )
                xraw.append(xt)
            if split:
                # device-side hi/lo decomposition: x = hi + lo, both f32r
                xins = {}
                for t in (0, 1):
                    hi = xin_pool.tile([128, Gc, 256], f32r, tag=f"xhi{t}", name=f"xhi{t}")
                    nc.scalar.copy(hi[:], xraw[t][:])
                    lo = xin_pool.tile([128, Gc, 256], f32r, tag=f"xlo{t}", name=f"xlo{t}")
                    nc.vector.tensor_sub(lo[:], xraw[t][:], hi[:])
                    xins[(t, 0)] = hi
                    xins[(t, 1)] = lo
            else:
                xins = {(t, 0): xraw[t] for t in (0, 1)}
            youts = {
                q: yout_pool.tile([128, Gc, 256], f32, tag=f"yout{q}", name=f"yout{q}")
                for q in (0, 1)
            }
            for j in range(Gc):
                vts = {}
                for m in (0, 1):
                    for r in range(rank):
                        p1 = p1_pool.tile([128, 256], f32, tag="p1")
                        mmops = [(t, s) for t in (0, 1) for s in parts]
                        for i, (t, s) in enumerate(mmops):
                            nc.tensor.matmul(
                                p1[:],
                                xins[(t, s)][:, j, m * 128:(m + 1) * 128],
                                bvt[r][t][:],
                                start=(i == 0),
                                stop=(i == len(mmops) - 1),
                            )
                        if split:
                            vhi = vt_pool.tile([128, 256], f32r,
                                               tag=f"vth{m}_{r}", name=f"vth{m}_{r}")
                            nc.scalar.copy(vhi[:], p1[:])
                            vlo = vt_pool.tile([128, 256], f32r,
                                               tag=f"vtl{m}_{r}", name=f"vtl{m}_{r}")
                            nc.vector.tensor_sub(vlo[:], p1[:], vhi[:])
                            vts[(r, m, 0)] = vhi
                            vts[(r, m, 1)] = vlo
                        else:
                            v = vt_pool.tile([128, 256], mmdt,
                                             tag=f"vt{m}_{r}", name=f"vt{m}_{r}")
                            if m == 0:
                                nc.vector.tensor_copy(v[:], p1[:])
                            else:
                                nc.scalar.copy(v[:], p1[:])
                            vts[(r, m, 0)] = v
                if pending[0] is not None:
                    emit_pass2(pending[0])
                pending[0] = (vts, youts, j, g)
        emit_pass2(pending[0])


def _build(rank, precision, reps=1, loop_reps=None, cfg=None):
    key = (rank, precision, reps, loop_reps,
           tuple(sorted((cfg or {}).items())))
    if key in _BUILD_CACHE:
        return _BUILD_CACHE[key]
    f32 = mybir.dt.float32
    mmdt = f32 if precision == "fp32" else mybir.dt.float32r
    xdt = f32 if precision in ("fp32", "fp32r_split") else mmdt
    nc = bacc.Bacc("TRN2", target_bir_lowering=False, debug=False)
    x = nc.dram_tensor("x", [C, H, W], xdt, kind="ExternalInput").ap()
    bv = nc.dram_tensor("bv", [rank, 2, 128, 256], mmdt, kind="ExternalInput").ap()
    bh = nc.dram_tensor("bh", [rank, 2, 128, 256], mmdt, kind="ExternalInput").ap()
    y = nc.dram_tensor("y", [C, H, W], f32, kind="ExternalOutput").ap()
    with TileContext(nc) as tc:
        with tc.tile_pool(name="bands", bufs=1) as band_pool:
            bvt = [[None, None] for _ in range(rank)]
            bht = [[None, None] for _ in range(rank)]
            for r in range(rank):
                for t in (0, 1):
                    bvt[r][t] = band_pool.tile([128, 256], mmdt, tag=f"bv{r}{t}", name=f"bv{r}{t}")
                    nc.sync.dma_start(out=bvt[r][t][:], in_=bv[r, t])
                    bht[r][t] = band_pool.tile([128, 256], mmdt, tag=f"bh{r}{t}", name=f"bh{r}{t}")
                    nc.sync.dma_start(out=bht[r][t][:], in_=bh[r, t])
            if loop_reps is not None:
                with tc.For_i(0, loop_reps, 1):
                    _emit(nc, tc, x, y, bvt, bht, rank, precision, cfg)
            else:
                for _ in range(reps):
                    _emit(nc, tc, x, y, bvt, bht, rank, precision, cfg)
    nc.compile()
    _BUILD_CACHE[key] = nc
    return nc


def _prep_inputs(fmap, kernel4x4, precision):
    comps = _factorize(kernel4x4)
    rank = max(1, len(comps))
    while len(comps) < rank:
        comps.append((np.zeros(4), np.zeros(4)))
    bv = np.zeros((rank, 2, 128, 256), dtype=np.float32)
    bh = np.zeros((rank, 2, 128, 256), dtype=np.float32)
    for r, (u, v) in enumerate(comps):
        Bv = _band(u, H).astype(np.float32)  # [y, y']
        Bh = _band(v, W).astype(np.float32)  # [x, x']
        bv[r] = Bv.reshape(2, 128, 256)
        bh[r] = Bh.reshape(2, 128, 256)
    if precision in ("fp32r", "fp32r_split"):
        bv, bh = _round_f32r(bv), _round_f32r(bh)
    in_maps = []
    for i in range(N_CORES):
        shard = np.ascontiguousarray(fmap[i], dtype=np.float32)
        if precision == "fp32r":
            shard = _round_f32r(shard)

        in_maps.append({"x": shard, "bv": bv, "bh": bh})
    return rank, in_maps


def kernel(fmap, kernel):
    fmap = np.asarray(fmap)
    kern = np.asarray(kernel)
    assert fmap.shape == (N_CORES, C, H, W), fmap.shape
    rank, in_maps = _prep_inputs(fmap, kern, PRECISION)
    nc = _build(rank, PRECISION)
    last_err = None
    for _attempt in range(3):
        try:
            res = run_bass_kernel_spmd(nc, in_maps, list(range(N_CORES)), trace=False)
            break
        except Exception as e:  # transient device wedge -> retry
            last_err = e
            import time
            time.sleep(2.0)
    else:
        raise last_err
    out = np.stack([res.results[i]["y"] for i in range(N_CORES)], axis=0)
    return np.ascontiguousarray(out, dtype=np.float32)


# revision 14
# speedup vs baseline: 1714.9658x; 1.0156x over previous
"""Trainium2 Bass kernel for nn_BlurF: depthwise 4x4 blur (upfirdn2d pad=(2,1)).

Strategy: data-parallel over batch (8 cores x 1 image of [128,256,256]).
Per core, the separable conv is computed as two PE banded-matmul passes
using the data as the stationary operand, which transposes each pass:
  pass1: VT[x, y'] = sum_y X[y, x] * Bv[y, y']   (vertical conv, transposed)
  pass2: OUT[y', x'] = sum_x VT[x, y'] * Bh[x, x'] (horizontal conv, back)
Boundary zero-padding is folded into the band matrices. Matmuls run in
float32r (inputs rounded to 12-bit significand; accumulation is fp32 in
PSUM). General (non-separable) 4x4 kernels are handled via SVD as a sum
of up to 4 separable components.
"""

import numpy as np
import concourse.bacc as bacc
import concourse.mybir as mybir
from concourse.tile import TileContext
from concourse.bass_utils import run_bass_kernel_spmd

N_CORES = 8
C, H, W = 128, 256, 256
PRECISION = "fp32r"  # "fp32r" (fast, ~1.3e-4 rel err) | "fp32r_split" (hi/lo, ~6e-8) | "fp32"

_BUILD_CACHE = {}


def _round_f32r(a):
    """Round fp32 array to float32r (11 stored mantissa bits), round-half-up."""
    b = np.ascontiguousarray(a, dtype=np.float32).view(np.uint32)
    b = (b + np.uint32(0x800)) & np.uint32(0xFFFFF000)
    return b.view(np.float32)


def _factorize(kernel4x4):
    """kernel[a,b] = sum_r u_r[a] v_r[b]; returns list of (u, v) float64."""
    k = np.asarray(kernel4x4, dtype=np.float64)
    U, S, Vt = np.linalg.svd(k)
    comps = []
    for r in range(4):
        if S[r] > 1e-9 * max(S[0], 1e-30):
            comps.append((U[:, r] * np.sqrt(S[r]), Vt[r, :] * np.sqrt(S[r])))
    return comps


def _band(taps, n):
    """B[s, s'] = taps[a] where s = s' + 1 - a, for a in 0..3, clipped to [0,n)."""
    B = np.zeros((n, n), dtype=np.float64)
    for a in range(4):
        # s' = s + a - 1
        lo = max(0, 1 - a)
        hi = min(n, n + 1 - a)
        s = np.arange(lo, hi)
        B[s, s + a - 1] = taps[a]
    return B


DEFAULT_CFG = dict(
    G=16, out_engine="scalar", dma_split=2,
    xin_bufs=2, vt_bufs=3, yout_bufs=2, p1_bufs=3, p2_bufs=3,
)


def _emit(nc, tc, x, y, bvt, bht, rank, precision, cfg=None):
    cfg = {**DEFAULT_CFG, **(cfg or {})}
    Gc = cfg["G"]
    f32 = mybir.dt.float32
    f32r = mybir.dt.float32r
    mmdt = f32 if precision == "fp32" else f32r
    split = precision == "fp32r_split"
    parts = (0, 1) if split else (0,)
    NG = C // Gc
    out_dma = nc.scalar if cfg["out_engine"] == "scalar" else nc.sync
    with (
        tc.tile_pool(name="xin", bufs=cfg["xin_bufs"]) as xin_pool,
        tc.tile_pool(name="vt", bufs=cfg["vt_bufs"]) as vt_pool,
        tc.tile_pool(name="yout", bufs=cfg["yout_bufs"]) as yout_pool,
        tc.tile_pool(name="p1", bufs=cfg["p1_bufs"], space="PSUM") as p1_pool,
        tc.tile_pool(name="p2", bufs=cfg["p2_bufs"], space="PSUM") as p2_pool,
    ):
        pending = [None]

        def emit_pass2(p):
            vts, youts, j, g = p
            ops = [(r, m, s) for r in range(rank) for m in (0, 1) for s in parts]
            for q in (0, 1):
                p2 = p2_pool.tile([128, 256], f32, tag="p2")
                for i, (r, m, s) in enumerate(ops):
                    nc.tensor.matmul(
                        p2[:],
                        vts[(r, m, s)][:, q * 128:(q + 1) * 128],
                        bht[r][m][:],
                        start=(i == 0),
                        stop=(i == len(ops) - 1),
                    )
                if q == 0:
                    nc.vector.tensor_copy(youts[q][:, j, :], p2[:])
                else:
                    nc.scalar.copy(youts[q][:, j, :], p2[:])
            ds = cfg["dma_split"]
            gsz = Gc // ds
            if (j + 1) % gsz == 0:
                h = (j + 1) // gsz - 1  # finished chunk index
                c0 = g * Gc + h * gsz
                for q in (0, 1):
                    out_dma.dma_start(
                        out=y[c0:c0 + gsz, q * 128:(q + 1) * 128, :]
                        .rearrange("c y x -> y c x"),
                        in_=youts[q][:, h * gsz:(h + 1) * gsz, :],
                    )

        for g in range(NG):
            xraw = []
            ds = cfg["dma_split"]
            gsz = Gc // ds
            for t in (0, 1):
                xt = xin_pool.tile([128, Gc, 256], f32 if split else mmdt,
                                   tag=f"xin{t}", name=f"xin{t}")
                for h in range(ds):
                    c0 = g * Gc + h * gsz
                    nc.sync.dma_start(
                        out=xt[:, h * gsz:(h + 1) * gsz, :],
                        in_=x[c0:c0 + gsz, t * 128:(t + 1) * 128, :]
                        .rearrange("c y x -> y c x"),
                    )
                xraw.append(xt)
            if split:
                # device-side hi/lo decomposition: x = hi + lo, both f32r
                xins = {}
                for t in (0, 1):
                    hi = xin_pool.tile([128, Gc, 256], f32r, tag=f"xhi{t}", name=f"xhi{t}")
                    nc.scalar.copy(hi[:], xraw[t][:])
                    lo = xin_pool.tile([128, Gc, 256], f32r, tag=f"xlo{t}", name=f"xlo{t}")
                    nc.vector.tensor_sub(lo[:], xraw[t][:], hi[:])
                    xins[(t, 0)] = hi
                    xins[(t, 1)] = lo
            else:
                xins = {(t, 0): xraw[t] for t in (0, 1)}
            youts = {
                q: yout_pool.tile([128, Gc, 256], f32, tag=f"yout{q}", name=f"yout{q}")
                for q in (0, 1)
            }
            for j in range(Gc):
                vts = {}
                for m in (0, 1):
                    for r in range(rank):
                        p1 = p1_pool.tile([128, 256], f32, tag="p1")
                        mmops = [(t, s) for t in (0, 1) for s in parts]
                        for i, (t, s) in enumerate(mmops):
                            nc.tensor.matmul(
                                p1[:],
                                xins[(t, s)][:, j, m * 128:(m + 1) * 128],
                                bvt[r][t][:],
                                start=(i == 0),
                                stop=(i == len(mmops) - 1),
                            )
                        if split:
                            vhi = vt_pool.tile([128, 256], f32r,
                                               tag=f"vth{m}_{r}", name=f"vth{m}_{r}")
                            nc.scalar.copy(vhi[:], p1[:])
                            vlo = vt_pool.tile([128, 256], f32r,
                                               tag=f"vtl{m}_{r}", name=f"vtl{m}_{r}")
                            nc.vector.tensor_sub(vlo[:], p1[:], vhi[:])
                            vts[(r, m, 0)] = vhi
                            vts[(r, m, 1)] = vlo
                        else:
                            v = vt_pool.tile([128, 256], mmdt,
                                             tag=f"vt{m}_{r}", name=f"vt{m}_{r}")
                            if m == 0:
                                nc.vector.tensor_copy(v[:], p1[:])
                            else:
                                nc.scalar.copy(v[:], p1[:])
                            vts[(r, m, 0)] = v
                if pending[0] is not None:
                    emit_pass2(pending[0])
                pending[0] = (vts, youts, j, g)
        emit_pass2(pending[0])


def _build(rank, precision, reps=1, loop_reps=None, cfg=None):
    key = (rank, precision, reps, loop_reps,
           tuple(sorted((cfg or {}).items())))
    if key in _BUILD_CACHE:
        return _BUILD_CACHE[key]
    f32 = mybir.dt.float32
    mmdt = f32 if precision == "fp32" else mybir.dt.float32r
    xdt = f32 if precision in ("fp32", "fp32r_split") else mmdt
    nc = bacc.Bacc("TRN2", target_bir_lowering=False, debug=False)
    x = nc.dram_tensor("x", [C, H, W], xdt, kind="ExternalInput").ap()
    bv = nc.dram_tensor("bv", [rank, 2, 128, 256], mmdt, kind="ExternalInput").ap()
    bh = nc.dram_tensor("bh", [rank, 2, 128, 256], mmdt, kind="ExternalInput").ap()
    y = nc.dram_tensor("y", [C, H, W], f32, kind="ExternalOutput").ap()
    with TileContext(nc) as tc:
        with tc.tile_pool(name="bands", bufs=1) as band_pool:
            bvt = [[None, None] for _ in range(rank)]
            bht = [[None, None] for _ in range(rank)]
            for r in range(rank):
                for t in (0, 1):
                    bvt[r][t] = band_pool.tile([128, 256], mmdt, tag=f"bv{r}{t}", name=f"bv{r}{t}")
                    nc.sync.dma_start(out=bvt[r][t][:], in_=bv[r, t])
                    bht[r][t] = band_pool.tile([128, 256], mmdt, tag=f"bh{r}{t}", name=f"bh{r}{t}")
                    nc.sync.dma_start(out=bht[r][t][:], in_=bh[r, t])
            if loop_reps is not None:
                with tc.For_i(0, loop_reps, 1):
                    _emit(nc, tc, x, y, bvt, bht, rank, precision, cfg)
            else:
                for _ in range(reps):
                    _emit(nc, tc, x, y, bvt, bht, rank, precision, cfg)
    nc.compile()
    _BUILD_CACHE[key] = nc
    return nc


def _prep_inputs(fmap, kernel4x4, precision):
    comps = _factorize(kernel4x4)
    rank = max(1, len(comps))
    while len(comps) < rank:
        comps.append((np.zeros(4), np.zeros(4)))
    bv = np.zeros((rank, 2, 128, 256), dtype=np.float32)
    bh = np.zeros((rank, 2, 128, 256), dtype=np.float32)
    for r, (u, v) in enumerate(comps):
        Bv = _band(u, H).astype(np.float32)  # [y, y']
        Bh = _band(v, W).astype(np.float32)  # [x, x']
        bv[r] = Bv.reshape(2, 128, 256)
        bh[r] = Bh.reshape(2, 128, 256)
    if precision in ("fp32r", "fp32r_split"):
        bv, bh = _round_f32r(bv), _round_f32r(bh)
    in_maps = []
    for i in range(N_CORES):
        shard = np.ascontiguousarray(fmap[i], dtype=np.float32)
        if precision == "fp32r":
            shard = _round_f32r(shard)

        in_maps.append({"x": shard, "bv": bv, "bh": bh})
    return rank, in_maps


def kernel(fmap, kernel):
    fmap = np.asarray(fmap)
    kern = np.asarray(kernel)
    assert fmap.shape == (N_CORES, C, H, W), fmap.shape
    rank, in_maps = _prep_inputs(fmap, kern, PRECISION)
    nc = _build(rank, PRECISION)
    last_err = None
    for _attempt in range(3):
        try:
            res = run_bass_kernel_spmd(nc, in_maps, list(range(N_CORES)), trace=False)
            break
        except Exception as e:  # transient device wedge -> retry
            last_err = e
            import time
            time.sleep(2.0)
    else:
        raise last_err
    out = np.stack([res.results[i]["y"] for i in range(N_CORES)], axis=0)
    return np.ascontiguousarray(out, dtype=np.float32)
